# revision 3
# baseline (speedup 1.0000x reference)
"""ContrastAwareAttentionBlock Trainium2 Bass kernel.

Sharding: 8 cores = (batch 4) x (image half: rows 0-63 / 64-127); each core
computes its half with a 6-row halo of redundant compute — no collectives.

Layout: channel-major [64ch (+2 indicator rows), pixels], rows padded to 130
cols. All matmuls float32r. Conv 3x3 = 9 PSUM-accumulated matmuls over
shifted slices of the same tile; BN folded into weights (scale) and an
indicator-channel matmul term (bias), so out-of-image pixels stay exactly 0
and one SPMD program serves both halves. ReLU fused into the ACT-engine PSUM
evacuation. Attention per 3-row chunk and patch-position p: logits matmul ->
exp on ACT (with -80 bias so invalid pixels give ~0) -> exp replicated to
(head,dim) channels via 0/1-selector matmuls (two q's at once into a
128-partition PSUM) -> DVE multiply against a [v; v shifted one col] stack ->
q-accumulate -> multiply by reciprocal of PE-computed replicated softmax
denominator (invalid pixels get a huge denominator via the inverse-indicator
channel, flushing them to 0). Fold+projection fused as 9 shifted
PSUM-accumulated matmuls over a 3-chunk xw ring buffer.
"""
import os
import sys
sys.path.insert(0, '/opt/trn_rl_repo')
import numpy as np

import concourse.bass as bass
import concourse.tile as tile
from concourse import mybir
from concourse.bass_utils import run_bass_kernel_spmd

F32 = mybir.dt.float32
F32R = mybir.dt.float32r
BF16 = mybir.dt.bfloat16
AF = mybir.ActivationFunctionType
OP = mybir.AluOpType

B, C, H, W, HEADS, HD = 4, 64, 128, 128, 8, 8
PW = W + 2
TROWS = 78            # tile rows map to image rows [s-7, s+71)
NPX = TROWS * PW
N_CORES = 8
BN_EPS = 1e-5
BIG = 80.0            # exp(-BIG) ~ 0 for invalid pixels
BIGD = 1e30           # denominator for invalid pixels
LARGE = 1e30          # relu clamp for invalid pixels
CONVN = 390           # conv matmul chunk (3 rows, 1 PSUM bank)
MAX_WAITS = 1

TAPS = [(di, dj) for di in (-1, 0, 1) for dj in (-1, 0, 1)]


def _split_excess_waits(nc):
    """This walrus build rejects >1 sync wait per instruction; move excess
    waits onto same-engine NOPs inserted before the offender."""
    bbs, fixups = [], {}
    for f in nc.m.functions:
        for bb in f.blocks:
            bbs.append(bb)
            for inst in bb.instructions:
                si = inst.sync_info
                waits = list(si.on_wait) if si is not None and si.on_wait else []
                if len(waits) > MAX_WAITS:
                    si.on_wait = waits[:MAX_WAITS]
                    rest = waits[MAX_WAITS:]
                    chunks = [rest[i:i + MAX_WAITS]
                              for i in range(0, len(rest), MAX_WAITS)]
                    fixups.setdefault(id(bb), {}).setdefault(inst.name, []).extend(
                        (inst.engine, ch) for ch in chunks)
    if not fixups:
        return
    created = {}
    for bb in bbs:
        for name, specs in fixups.get(id(bb), {}).items():
            nops = []
            for engine, ch in specs:
                bi = nc.engines[engine].nop(nofuse=True)
                bi.ins.sync_info = mybir.SyncInfo(on_wait=ch, on_update=[])
                nops.append(bi.ins)
            created[name] = nops
    all_nops = {n.name for ns in created.values() for n in ns}
    for bb in bbs:
        insts = [i for i in bb.instructions if i.name not in all_nops]
        new = []
        for inst in insts:
            new.extend(created.get(inst.name, ()))
            new.append(inst)
        bb.instructions = new


class _TCtx(tile.TileContext):
    def _drain_and_barrier(self, tick_clock, wait_clock):
        from concourse.tile import ScopedClock
        probe = self.nc.sync.nop(nofuse=True)
        wait_clock.add_sem_waits(
            probe.ins, ScopedClock({None: tick_clock.global_clock}))
        self.nc.sync.drain()
        self.nc.all_engine_barrier()
        assert self.sems is not None
        popped = self.nc._tile_sem_poison_stack.pop()
        assert popped is self._sem_poison
        self.nc.clear_and_free_semaphores(list(self.sems.allocated().values()))
        self.nc.all_engine_barrier()

    def __exit__(self, exc_type, exc_val, exc_tb):
        ret = super().__exit__(exc_type, exc_val, exc_tb)
        if exc_type is None:
            _split_excess_waits(self.nc)
        return ret


def _prep_weights(conv_w, bn_g, bn_b, bn_m, bn_v, w_v, b_v, w_attn, b_attn,
                  w_proj, b_proj):
    sc = HD ** -0.5
    inv = bn_g / np.sqrt(bn_v + BN_EPS)          # [4, 64]
    beta = bn_b - bn_m * inv                     # [4, 64]
    cols, parts = {}, []

    def add(name, a):
        a = np.asarray(a, np.float32)
        full = np.zeros((72, a.shape[1]), np.float32)
        full[:a.shape[0]] = a
        cols[name] = sum(p.shape[1] for p in parts)
        parts.append(full)

    # conv: per (conv k, tap): lhsT [65, 64]; inv folded into W columns;
    # beta on indicator row 64 for the center tap only.
    wc = np.zeros((66, 4 * 9 * 64), np.float32)
    for k in range(4):
        for t, (di, dj) in enumerate(TAPS):
            blk = (k * 9 + t) * 64
            # out[d] += sum_c W[d,c,di+1,dj+1]*inv[d] * x[c]
            wc[:64, blk:blk + 64] = conv_w[k, :, :, di + 1, dj + 1].T * inv[k][None, :]
            if (di, dj) == (0, 0):
                wc[64, blk:blk + 64] = beta[k]
                wc[65, blk:blk + 64] = -LARGE
    add("conv", wc)
    wv65 = np.zeros((65, 64), np.float32)
    wv65[:64] = w_v.T
    wv65[64] = b_v
    add("wv", wv65)
    # attention logits, channel order per p: (q, n); prescaled; bias on the
    # indicator row, -BIG on the inverse-indicator row (so invalid pixels
    # get exp(-BIG) ~ 0 with no extra ACT bias constant).
    wa = np.zeros((66, 9 * 72), np.float32)
    for p in range(9):
        for q in range(9):
            for n in range(HEADS):
                row = n * 81 + p * 9 + q
                wa[:64, p * 72 + q * 8 + n] = w_attn[row] * sc
                wa[64, p * 72 + q * 8 + n] = b_attn[row] * sc
                wa[65, p * 72 + q * 8 + n] = -BIG
    add("wa", wa)
    selp = np.zeros((72, 3 * 128), np.float32)
    sels = np.zeros((72, 3 * 64), np.float32)
    den = np.zeros((72, 64), np.float32)
    for q in range(9):
        k, r = divmod(q, 3)
        for n in range(HEADS):
            for d in range(HD):
                if r < 2:
                    selp[q * 8 + n, k * 128 + r * 64 + n * 8 + d] = 1.0
                else:
                    sels[q * 8 + n, k * 64 + n * 8 + d] = 1.0
                den[q * 8 + n, n * 8 + d] = 1.0
    add("selp", selp)
    add("sels", sels)
    add("den", den)
    bigd = np.zeros((66, 64), np.float32)
    bigd[65] = BIGD                              # inverse-indicator row
    add("bigd", bigd)
    idm = np.zeros((72, 64), np.float32)
    idm[:64] = np.eye(64, dtype=np.float32)
    add("id64", idm)
    add("wp", w_proj.T)
    bpi = np.zeros((66, 64), np.float32)
    bpi[64] = b_proj                             # indicator row
    bpi[65] = -LARGE
    add("bpi", bpi)
    bpineg = np.zeros((66, 64), np.float32)
    bpineg[64] = -b_proj
    bpineg[65] = -LARGE
    add("bpineg", bpineg)
    # fold128: [128, 64], sums upper+lower 64-halves
    fold = np.zeros((128, 64), np.float32)
    fold[:64] = np.eye(64, dtype=np.float32)
    fold[64:] = np.eye(64, dtype=np.float32)
    return np.concatenate(parts, axis=1), cols, fold


def _build(wcols, wtotal):
    nc = bass.Bass("TRN2", target_bir_lowering=False, debug=False)
    xs = nc.dram_tensor("xs", [66, NPX], F32, kind="ExternalInput").ap()
    wts = nc.dram_tensor("wts", [72, wtotal], F32, kind="ExternalInput").ap()
    wts2 = nc.dram_tensor("wts2", [128, 64], F32, kind="ExternalInput").ap()
    y = nc.dram_tensor("y", [C, 64 * W], F32, kind="ExternalOutput").ap()

    def wr_at(w, name, r0, r1, n=64):
        return w[r0:r1, wcols[name]:wcols[name] + n]

    with _TCtx(nc) as tc:
        cpool = tc.alloc_tile_pool(name="const", bufs=1)
        spool = tc.alloc_tile_pool(name="stage", bufs=2)
        rpool_ = tc.alloc_tile_pool(name="raw", bufs=1, side="right")

        wf = rpool_.tile([72, wtotal], F32, tag="wf")
        nc.sync.dma_start(wf[:], wts[:])
        wr = cpool.tile([72, wtotal], F32R)
        nc.vector.tensor_copy(wr[:], wf[:])
        w2f = rpool_.tile([128, 64], F32, tag="w2f")
        nc.sync.dma_start(w2f[:], wts2[:])
        fold128 = cpool.tile([128, 64], BF16)
        nc.vector.tensor_copy(fold128[:], w2f[:])
        wb = cpool.tile([72, 192], BF16)
        nc.vector.tensor_copy(wb[:, 0:64], wf[:, wcols["wp"]:wcols["wp"] + 64])
        nc.vector.tensor_copy(wb[:, 64:128],
                              wf[:, wcols["id64"]:wcols["id64"] + 64])
        nc.vector.tensor_scalar_mul(wb[:, 128:192],
                                    wf[:, wcols["wp"]:wcols["wp"] + 64], -1.0)

        def conv_w_ap(k, t):
            o = wcols["conv"] + (k * 9 + t) * 64
            return wr[0:66 if t == 4 else 65, o:o + 64]

        def conv_stage(cps, src, dst, k, r0, r1):
            lo, hi = r0 * PW + 1, r1 * PW - 1
            for o in range(lo, hi, CONVN):
                n = min(CONVN, hi - o)
                ps = cps.tile([64, CONVN], F32)
                for t, (di, dj) in enumerate(TAPS):
                    off = di * PW + dj
                    nc.tensor.matmul(ps[:, :n], conv_w_ap(k, t),
                                     src[0:66 if t == 4 else 65,
                                         o + off:o + off + n],
                                     start=(t == 0), stop=(t == 8))
                nc.scalar.activation(dst[0:64, o:o + n], ps[:, :n], AF.Relu)
            d3 = (dst[0:64].bitcast(F32)
                  .rearrange("c (r w) -> c r w", r=TROWS, w=PW))
            nc.gpsimd.memset(d3[:, r0:r1, 0:1], 0.0)
            nc.gpsimd.memset(d3[:, r0:r1, PW - 1:PW], 0.0)

        def ind_copy(dst, src):
            nc.sync.dma_start(dst[64:66, :], src[64:66, :])

        # ---- input & convs 1,2 ----
        x0f = rpool_.tile([66, NPX], F32, tag="x0f")
        nc.sync.dma_start(x0f[:], xs[:])
        x0 = spool.tile([66, NPX], F32R, tag="stage")
        nc.vector.tensor_copy(x0[:], x0f[:])
        rpool_.release()
        x1 = spool.tile([66, NPX], F32R, tag="stage")
        ind_copy(x1, x0)
        x2 = spool.tile([66, NPX], F32R, tag="stage")
        with tc.tile_pool(name="cps", bufs=4, space="PSUM") as cps:
            conv_stage(cps, x0, x1, 0, 2, 76)
            ind_copy(x2, x1)
            conv_stage(cps, x1, x2, 1, 3, 75)

        # ---- attention ----
        z = spool.tile([66, NPX], F32R, tag="stage")
        ind_copy(z, x2)

        GR0, GR1 = 4, 74
        ZR0, ZR1 = 5, 73
        n_chunks = (GR1 - GR0 + 2) // 3

        vpool = tc.alloc_tile_pool(name="vb", bufs=2)
        epool = tc.alloc_tile_pool(name="ep", bufs=2)
        rpool = tc.alloc_tile_pool(name="rd", bufs=2)
        tpool = tc.alloc_tile_pool(name="tmp", bufs=2)
        xwpool = tc.alloc_tile_pool(name="xw", bufs=3)
        vps = tc.alloc_tile_pool(name="vps", bufs=1, space="PSUM")
        lps = tc.alloc_tile_pool(name="lps", bufs=1, space="PSUM")
        dps = tc.alloc_tile_pool(name="dps", bufs=1, space="PSUM")
        apool = tc.alloc_tile_pool(name="aps", bufs=2, space="PSUM")
        fps = tc.alloc_tile_pool(name="fps", bufs=1, space="PSUM")
        yps = tc.alloc_tile_pool(name="yps", bufs=1, space="PSUM")

        xw_tiles = {}

        def chunk_rows(ci):
            t0 = GR0 + 3 * ci
            return t0, min(t0 + 3, GR1)

        def do_products(ci):
            t0, t1 = chunk_rows(ci)
            S0 = t0 * PW + 1
            SN = (t1 - t0) * PW - 2
            vbase = (t0 - 1) * PW
            vlen = (t1 - t0 + 2) * PW
            vb = vpool.tile([128, 5 * PW], F32, tag="vb")
            for j in range(2):
                vo = j * 390
                vn = min(390, vlen - vo)
                if vn <= 0:
                    continue
                psv = vps.tile([64, 390], F32, tag="v")
                nc.tensor.matmul(psv[:, :vn], wr_at(wr, "wv", 0, 65),
                                 x2[0:65, vbase + vo:vbase + vo + vn],
                                 start=True, stop=True)
                nc.scalar.copy(vb[0:64, vo:vo + vn], psv[:, :vn])
                if vo == 0:
                    nc.scalar.copy(vb[64:128, 0:vn - 1], psv[:, 1:vn])
                else:
                    nc.scalar.copy(vb[64:128, vo - 1:vo - 1 + vn], psv[:, :vn])
            for p in range(9):
                psl = lps.tile([72, CONVN], F32, tag="l")
                nc.tensor.matmul(psl[:, :SN],
                                 wr[0:66, wcols["wa"] + p * 72:
                                    wcols["wa"] + p * 72 + 72],
                                 x2[0:66, S0:S0 + SN], start=True, stop=True)
                ep = epool.tile([72, CONVN], F32R, tag="e")
                nc.scalar.activation(ep[:, :SN], psl[:, :SN], AF.Exp)
                psd = dps.tile([64, CONVN], F32, tag="d")
                nc.tensor.matmul(psd[:, :SN], wr_at(wr, "den", 0, 72),
                                 ep[:, :SN], start=True, stop=False)
                nc.tensor.matmul(psd[:, :SN], wr_at(wr, "bigd", 64, 66),
                                 x2[64:66, S0:S0 + SN], start=False, stop=True)
                rd = rpool.tile([64, CONVN], F32, tag="r")
                nc.vector.reciprocal(rd[:, :SN], psd[:, :SN])

                tM, tS = [], []
                for k in range(3):
                    psp = apool.tile([128, CONVN], F32, tag="a")
                    nc.tensor.matmul(psp[:, :SN],
                                     wr[0:72, wcols["selp"] + k * 128:
                                        wcols["selp"] + k * 128 + 128],
                                     ep[:, :SN], start=True, stop=True)
                    tm = tpool.tile([128, CONVN], BF16, tag=f"tM{k}")
                    vlo = S0 - vbase + (k - 1) * PW - 1
                    nc.vector.tensor_tensor(tm[:, :SN], psp[:, :SN],
                                            vb[0:128, vlo:vlo + SN], OP.mult)
                    tM.append(tm)
                    pss = apool.tile([128, CONVN], F32, tag="a")
                    nc.tensor.matmul(pss[0:64, :SN],
                                     wr[0:72, wcols["sels"] + k * 64:
                                        wcols["sels"] + k * 64 + 64],
                                     ep[:, :SN], start=True, stop=True)
                    ts = tpool.tile([64, CONVN], BF16, tag=f"tS{k}")
                    vso = S0 - vbase + (k - 1) * PW + 1
                    nc.vector.tensor_tensor(ts[:, :SN], pss[0:64, :SN],
                                            vb[0:64, vso:vso + SN], OP.mult)
                    tS.append(ts)
                psf = fps.tile([64, CONVN], F32, tag="f")
                for k in range(3):
                    nc.tensor.matmul(psf[:, :SN], fold128[:, :],
                                     tM[k][:, :SN], start=(k == 0), stop=False)
                for k in range(3):
                    nc.tensor.matmul(psf[:, :SN], wb[0:64, 64:128],
                                     tS[k][:, :SN], start=False, stop=(k == 2))
                xw = xwpool.tile([64, 3 * PW], BF16, tag=f"xw{p}")
                nc.vector.tensor_tensor(xw[:, 1:1 + SN], psf[:, :SN],
                                        rd[:, :SN], OP.mult)
                xwf = xw[:].bitcast(mybir.dt.uint16)
                nc.gpsimd.memset(xwf[:, 0:1], 0.0)
                nc.gpsimd.memset(xwf[:, 3 * PW - 1:3 * PW], 0.0)
                if t1 - t0 < 3:
                    nc.gpsimd.memset(xwf[:, (t1 - t0) * PW:3 * PW], 0.0)
                xw_tiles[(p, ci)] = (xw, t0)

        def do_proj(ci):
            t0, t1 = chunk_rows(ci)
            zr0, zr1 = max(t0, ZR0), min(t1, ZR1)
            if zr0 >= zr1:
                return
            zS0 = zr0 * PW + 1
            zSN = (zr1 - zr0) * PW - 2
            def accum(psy, wcol, bias_name):
                first = True
                for p, (di, dj) in enumerate(TAPS):
                    off = di * PW + dj
                    a, b = zS0 - off, zS0 - off + zSN
                    for cj in (ci - 1, ci, ci + 1):
                        if (p, cj) not in xw_tiles:
                            continue
                        xw, xt0 = xw_tiles[(p, cj)]
                        lo, hi = xt0 * PW, xt0 * PW + 3 * PW
                        pa, pb = max(a, lo), min(b, hi)
                        if pa >= pb:
                            continue
                        nc.tensor.matmul(psy[:, pa - a:pb - a],
                                         wb[0:64, wcol:wcol + 64],
                                         xw[0:64, pa - lo:pb - lo],
                                         start=first, stop=False)
                        first = False
                nc.tensor.matmul(psy[:, :zSN], wr_at(wr, bias_name, 64, 66),
                                 x2[64:66, zS0:zS0 + zSN],
                                 start=False, stop=True)

            psy = yps.tile([64, CONVN], F32, tag="y")
            accum(psy, 0, "bpi")
            # rows [ZR0, 7) and [71, ZR1) may be out-of-image (which side
            # depends on the half); evacuate those via
            # relu(psy - L*invind) - relu(-psy - L*invind) which is psy for
            # valid pixels and exactly 0 for invalid ones.
            edge = [(r0, r1) for (r0, r1) in ((zr0, min(zr1, 7)),
                                              (max(zr0, 71), zr1))
                    if r0 < r1]
            if edge:
                psyn = yps.tile([64, CONVN], F32, tag="yn")
                accum(psyn, 128, "bpineg")
                for (r0, r1) in edge:
                    e0 = r0 * PW + 1 - zS0
                    en = (r1 - r0) * PW - 2
                    tp = tpool.tile([64, CONVN], F32, tag="zp")
                    nc.scalar.activation(tp[:, :en], psy[:, e0:e0 + en],
                                         AF.Relu)
                    tn = tpool.tile([64, CONVN], F32, tag="zn")
                    nc.scalar.activation(tn[:, :en], psyn[:, e0:e0 + en],
                                         AF.Relu)
                    nc.vector.tensor_tensor(
                        z[0:64, r0 * PW + 1:r0 * PW + 1 + en],
                        tp[:, :en], tn[:, :en], OP.subtract)
            m0, m1 = max(zr0, 7), min(zr1, 71)
            if m0 < m1:
                e0 = m0 * PW + 1 - zS0
                en = (m1 - m0) * PW - 2
                nc.scalar.copy(z[0:64, m0 * PW + 1:m0 * PW + 1 + en],
                               psy[:, e0:e0 + en])

        for ci in range(n_chunks):
            do_products(ci)
            if ci >= 1:
                do_proj(ci - 1)
        do_proj(n_chunks - 1)
        z3 = z[0:64].bitcast(F32).rearrange("c (r w) -> c r w", r=TROWS, w=PW)
        nc.gpsimd.memset(z3[:, ZR0:ZR1, 0:1], 0.0)
        nc.gpsimd.memset(z3[:, ZR0:ZR1, PW - 1:PW], 0.0)

        for pool in (yps, fps, apool, dps, lps, vps, xwpool, tpool, rpool,
                     epool, vpool):
            pool.release()

        # ---- convs 3,4 and output ----
        y3 = spool.tile([66, NPX], F32R, tag="stage")
        ind_copy(y3, z)
        with tc.tile_pool(name="cps2", bufs=4, space="PSUM") as cps2:
            conv_stage(cps2, z, y3, 2, 6, 72)
            y4 = spool.tile([66, NPX], F32, tag="stage")
            conv_stage(cps2, y3, y4, 3, 7, 71)
        ysrc = y4[0:64].rearrange("c (r w) -> c r w", r=TROWS, w=PW)
        nc.sync.dma_start(y[:].rearrange("c (r w) -> c r w", r=64, w=W),
                          ysrc[:, 7:71, 1:1 + W])
        spool.release()
        cpool.release()
    return nc


_CACHE = {}


def kernel(**inputs):
    x = np.asarray(inputs["x"], np.float32)
    conv_w = np.asarray(inputs["conv_w"], np.float32)
    args = (conv_w, np.asarray(inputs["bn_g"], np.float32),
            np.asarray(inputs["bn_b"], np.float32),
            np.asarray(inputs["bn_m"], np.float32),
            np.asarray(inputs["bn_v"], np.float32),
            np.asarray(inputs["w_v"], np.float32),
            np.asarray(inputs["b_v"], np.float32),
            np.asarray(inputs["w_attn"], np.float32),
            np.asarray(inputs["b_attn"], np.float32),
            np.asarray(inputs["w_proj"], np.float32),
            np.asarray(inputs["b_proj"], np.float32))
    wts, wcols, fold = _prep_weights(*args)

    # per-core input shards with indicator channels
    shards = []
    for core in range(N_CORES):
        bb, half = divmod(core, 2)
        s = half * 64
        rowbase = s - 7
        sh = np.zeros((66, TROWS, PW), np.float32)
        r0, r1 = max(0, rowbase + 1), min(H, rowbase + 77)
        sh[:64, r0 - rowbase:r1 - rowbase, 1:1 + W] = x[bb, :, r0:r1, :]
        sh[64, r0 - rowbase:r1 - rowbase, 1:1 + W] = 1.0
        sh[65] = 1.0 - sh[64]
        shards.append(sh.reshape(66, NPX))

    key = "k1"
    if key not in _CACHE:
        _CACHE[key] = _build(wcols, wts.shape[1])
    nc = _CACHE[key]
    in_maps = [{"xs": shards[i], "wts": wts, "wts2": fold}
               for i in range(N_CORES)]
    tkw = {}
    if os.environ.get("KERNEL_TRACE"):
        tkw = dict(trace=True, tmpdir=os.environ.get("KERNEL_TRACE_DIR"))
    res = run_bass_kernel_spmd(nc, in_maps, core_ids=list(range(N_CORES)),
                               **tkw)
    global LAST_RESULT
    LAST_RESULT = res
    out = np.zeros((B, C, H, W), np.float32)
    for core in range(N_CORES):
        bb, half = divmod(core, 2)
        s = half * 64
        out[bb, :, s:s + 64, :] = res.results[core]["y"].reshape(C, 64, W)
    return out



# revision 7
# speedup vs baseline: 1.1319x; 1.1319x over previous
"""ContrastAwareAttentionBlock Trainium2 Bass kernel.

Sharding: 8 cores = (batch 4) x (image half: rows 0-63 / 64-127); each core
computes its half with a halo of redundant compute — no collectives.

Layout: channel-major [64ch, pixels], rows padded to 130 cols. v2 design:
no indicator channels — biases ride the ACT engine's per-partition bias
operand, and out-of-image rows are zeroed via a tiny per-core 0/1 row mask
(broadcast-multiplied on row-aligned tiles). Softmax normalization uses the
log-sum-exp form (s = exp(l - ln(sum exp l)) on ACT) instead of the DVE
reciprocal. The 9 softmax positions q are packed as four 128-row pairs
((0,1),(3,4),(6,7) against [v; v<<1col] and (2,5) against [v; v<<1row])
plus a single (8), so selector+fold each take 5 matmuls per (p,chunk)
instead of 6, and the denominator one instead of two.
"""
import os
import sys
sys.path.insert(0, '/opt/trn_rl_repo')
import numpy as np

import concourse.bass as bass
import concourse.tile as tile
from concourse import mybir
from concourse.bass import broadcast_tensor_aps
from concourse.bass_utils import run_bass_kernel_spmd

F32 = mybir.dt.float32
F32R = mybir.dt.float32r
BF16 = mybir.dt.bfloat16
AF = mybir.ActivationFunctionType
OP = mybir.AluOpType

B, C, H, W, HEADS, HD = 4, 64, 128, 128, 8, 8
PW = W + 2
TROWS = 78            # tile rows map to image rows [s-7, s+71)
NPX = TROWS * PW
N_CORES = 8
BN_EPS = 1e-5
CONVN = 390           # conv matmul chunk (3 rows, 1 PSUM bank)
MAX_WAITS = 1

TAPS = [(di, dj) for di in (-1, 0, 1) for dj in (-1, 0, 1)]


def _split_excess_waits(nc):
    """This walrus build rejects >1 sync wait per instruction; move excess
    waits onto same-engine NOPs inserted before the offender."""
    bbs, fixups = [], {}
    for f in nc.m.functions:
        for bb in f.blocks:
            bbs.append(bb)
            for inst in bb.instructions:
                si = inst.sync_info
                waits = list(si.on_wait) if si is not None and si.on_wait else []
                if len(waits) > MAX_WAITS:
                    si.on_wait = waits[:MAX_WAITS]
                    rest = waits[MAX_WAITS:]
                    chunks = [rest[i:i + MAX_WAITS]
                              for i in range(0, len(rest), MAX_WAITS)]
                    fixups.setdefault(id(bb), {}).setdefault(inst.name, []).extend(
                        (inst.engine, ch) for ch in chunks)
    if not fixups:
        return
    created = {}
    for bb in bbs:
        for name, specs in fixups.get(id(bb), {}).items():
            nops = []
            for engine, ch in specs:
                bi = nc.engines[engine].nop(nofuse=True)
                bi.ins.sync_info = mybir.SyncInfo(on_wait=ch, on_update=[])
                nops.append(bi.ins)
            created[name] = nops
    all_nops = {n.name for ns in created.values() for n in ns}
    for bb in bbs:
        insts = [i for i in bb.instructions if i.name not in all_nops]
        new = []
        for inst in insts:
            new.extend(created.get(inst.name, ()))
            new.append(inst)
        bb.instructions = new


class _TCtx(tile.TileContext):
    def _drain_and_barrier(self, tick_clock, wait_clock):
        from concourse.tile import ScopedClock
        probe = self.nc.sync.nop(nofuse=True)
        wait_clock.add_sem_waits(
            probe.ins, ScopedClock({None: tick_clock.global_clock}))
        self.nc.sync.drain()
        self.nc.all_engine_barrier()
        assert self.sems is not None
        popped = self.nc._tile_sem_poison_stack.pop()
        assert popped is self._sem_poison
        self.nc.clear_and_free_semaphores(list(self.sems.allocated().values()))
        self.nc.all_engine_barrier()

    def __exit__(self, exc_type, exc_val, exc_tb):
        ret = super().__exit__(exc_type, exc_val, exc_tb)
        if exc_type is None:
            _split_excess_waits(self.nc)
        return ret


def _prep_weights(conv_w, bn_g, bn_b, bn_m, bn_v, w_v, b_v, w_attn, b_attn,
                  w_proj, b_proj):
    sc = HD ** -0.5
    inv = bn_g / np.sqrt(bn_v + BN_EPS)          # [4, 64]
    beta = bn_b - bn_m * inv                     # [4, 64]
    cols, parts = {}, []

    def add(name, a):
        a = np.asarray(a, np.float32)
        full = np.zeros((72, a.shape[1]), np.float32)
        full[:a.shape[0]] = a
        cols[name] = sum(p.shape[1] for p in parts)
        parts.append(full)

    # conv: per (conv k, tap): lhsT [64, 64]; inv folded into W columns.
    wc = np.zeros((64, 4 * 9 * 64), np.float32)
    for k in range(4):
        for t, (di, dj) in enumerate(TAPS):
            blk = (k * 9 + t) * 64
            wc[:, blk:blk + 64] = conv_w[k, :, :, di + 1, dj + 1].T * inv[k][None, :]
    add("conv", wc)
    add("beta", beta.T)                          # [64, 4]
    add("wv", w_v.T)                             # [64, 64]
    add("bv", b_v[:, None])                      # [64, 1]
    # attention logits, channel order per p: (q, n); prescaled by sc.
    wa = np.zeros((64, 9 * 72), np.float32)
    ba = np.zeros((72, 9), np.float32)
    for p in range(9):
        for q in range(9):
            for n in range(HEADS):
                row = n * 81 + p * 9 + q
                wa[:, p * 72 + q * 8 + n] = w_attn[row] * sc
                ba[q * 8 + n, p] = b_attn[row] * sc
    add("wa", wa)
    add("ba", ba)
    # q-pair selectors: j=0..2 pairs (3j, 3j+1); j=3 pair (2, 5); single 8.
    selp = np.zeros((72, 4 * 128), np.float32)
    for j, (qa, qb) in enumerate(((0, 1), (3, 4), (6, 7), (2, 5))):
        for n in range(HEADS):
            for d in range(HD):
                selp[qa * 8 + n, j * 128 + n * 8 + d] = 1.0
                selp[qb * 8 + n, j * 128 + 64 + n * 8 + d] = 1.0
    add("selp", selp)
    sels8 = np.zeros((72, 64), np.float32)
    for n in range(HEADS):
        for d in range(HD):
            sels8[8 * 8 + n, n * 8 + d] = 1.0
    add("sels8", sels8)
    # den72: [72,72] out (q,n) = sum_q' in (q',n)
    den = np.zeros((72, 72), np.float32)
    for qo in range(9):
        for n in range(HEADS):
            for qi in range(9):
                den[qi * 8 + n, qo * 8 + n] = 1.0
    add("den", den)
    idm = np.zeros((64, 64), np.float32)
    idm[:64] = np.eye(64, dtype=np.float32)
    add("id64", idm)
    add("wp", w_proj.T)
    add("bp", b_proj[:, None])
    # fold128: [128, 64], sums upper+lower 64-halves
    fold = np.zeros((128, 64), np.float32)
    fold[:64] = np.eye(64, dtype=np.float32)
    fold[64:] = np.eye(64, dtype=np.float32)
    return np.concatenate(parts, axis=1), cols, fold


def _build(wcols, wtotal):
    nc = bass.Bass("TRN2", target_bir_lowering=False, debug=False)
    xs = nc.dram_tensor("xs", [64, NPX], F32, kind="ExternalInput").ap()
    wts = nc.dram_tensor("wts", [72, wtotal], F32, kind="ExternalInput").ap()
    wts2 = nc.dram_tensor("wts2", [128, 64], F32, kind="ExternalInput").ap()
    msk = nc.dram_tensor("msk", [72, TROWS], F32, kind="ExternalInput").ap()
    y = nc.dram_tensor("y", [C, 64 * W], F32, kind="ExternalOutput").ap()

    def wr_at(w, name, r0, r1, n=64):
        return w[r0:r1, wcols[name]:wcols[name] + n]

    with _TCtx(nc) as tc:
        cpool = tc.alloc_tile_pool(name="const", bufs=1)
        spool = tc.alloc_tile_pool(name="stage", bufs=2)
        rpool_ = tc.alloc_tile_pool(name="raw", bufs=1, side="right")

        wf = rpool_.tile([72, wtotal], F32, tag="wf")
        nc.sync.dma_start(wf[:], wts[:])
        wr = cpool.tile([72, wtotal], F32R)
        nc.vector.tensor_copy(wr[:], wf[:])
        # biases stay f32 (copy of the columns we need)
        NB = 4 + 1 + 9 + 1
        bia = cpool.tile([72, NB], F32)
        nc.vector.tensor_copy(bia[0:64, 0:4], wf[0:64, wcols["beta"]:wcols["beta"] + 4])
        nc.vector.tensor_copy(bia[0:64, 4:5], wf[0:64, wcols["bv"]:wcols["bv"] + 1])
        nc.vector.tensor_copy(bia[0:72, 5:14], wf[0:72, wcols["ba"]:wcols["ba"] + 9])
        nc.vector.tensor_copy(bia[0:64, 14:15], wf[0:64, wcols["bp"]:wcols["bp"] + 1])

        def bbeta(k):
            return bia[0:64, k:k + 1]

        bvv = bia[0:64, 4:5]
        bpp = bia[0:64, 14:15]

        def bba(p):
            return bia[0:72, 5 + p:6 + p]

        w2f = rpool_.tile([128, 64], F32, tag="w2f")
        nc.sync.dma_start(w2f[:], wts2[:])
        fold128 = cpool.tile([128, 64], BF16)
        nc.vector.tensor_copy(fold128[:], w2f[:])
        wb = cpool.tile([72, 128], BF16)
        nc.vector.tensor_copy(wb[:, 0:64], wf[:, wcols["wp"]:wcols["wp"] + 64])
        nc.vector.tensor_copy(wb[:, 64:128],
                              wf[:, wcols["id64"]:wcols["id64"] + 64])
        mskf = cpool.tile([72, TROWS], F32)
        mf_ = rpool_.tile([72, TROWS], F32, tag="mf")
        nc.sync.dma_start(mf_[:], msk[:])
        nc.vector.tensor_copy(mskf[:], mf_[:])

        def conv_w_ap(k, t):
            o = wcols["conv"] + (k * 9 + t) * 64
            return wr[0:64, o:o + 64]

        def mask_rows(dst, r0, r1):
            # dst3 rows [r0, r1) *= msk row value (broadcast over PW)
            if r0 >= r1:
                return
            d3 = dst[0:64].rearrange("c (r w) -> c r w", r=TROWS, w=PW)
            a = d3[:, r0:r1, :]
            m = mskf[0:64, r0:r1].rearrange("c (r u) -> c r u", u=1)
            a2, m2 = broadcast_tensor_aps(a, m)
            nc.vector.tensor_tensor(a2, a2, m2, OP.mult)

        def conv_stage(cps, src, dst, k, r0, r1):
            lo, hi = r0 * PW + 1, r1 * PW - 1
            for o in range(lo, hi, CONVN):
                n = min(CONVN, hi - o)
                ps = cps.tile([64, CONVN], F32)
                for t, (di, dj) in enumerate(TAPS):
                    off = di * PW + dj
                    nc.tensor.matmul(ps[:, :n], conv_w_ap(k, t),
                                     src[0:64, o + off:o + off + n],
                                     start=(t == 0), stop=(t == 8))
                nc.scalar.activation(dst[0:64, o:o + n], ps[:, :n], AF.Relu,
                                     bias=bbeta(k))
            mask_rows(dst, r0, 7)
            mask_rows(dst, 71, r1)
            d3 = (dst[0:64].bitcast(F32)
                  .rearrange("c (r w) -> c r w", r=TROWS, w=PW))
            nc.gpsimd.memset(d3[:, r0:r1, 0:1], 0.0)
            nc.gpsimd.memset(d3[:, r0:r1, PW - 1:PW], 0.0)

        # ---- input & convs 1,2 ----
        x0f = rpool_.tile([64, NPX], F32, tag="x0f")
        nc.sync.dma_start(x0f[:], xs[:])
        x0 = spool.tile([64, NPX], F32R, tag="stage")
        nc.vector.tensor_copy(x0[:], x0f[:])
        rpool_.release()
        x1 = spool.tile([64, NPX], F32R, tag="stage")
        x2 = spool.tile([64, NPX], F32R, tag="stage")
        with tc.tile_pool(name="cps", bufs=4, space="PSUM") as cps:
            conv_stage(cps, x0, x1, 0, 2, 76)
            conv_stage(cps, x1, x2, 1, 3, 75)

        # ---- attention ----
        z = spool.tile([64, NPX], F32R, tag="stage")

        GR0, GR1 = 4, 74
        ZR0, ZR1 = 5, 73
        n_chunks = (GR1 - GR0 + 2) // 3

        vpool = tc.alloc_tile_pool(name="vb", bufs=2)
        epool = tc.alloc_tile_pool(name="ep", bufs=2)
        lpool = tc.alloc_tile_pool(name="lse", bufs=2)
        spool2 = tc.alloc_tile_pool(name="sm", bufs=2)
        tpool = tc.alloc_tile_pool(name="tmp", bufs=2)
        xwpool = tc.alloc_tile_pool(name="xw", bufs=3)
        vps = tc.alloc_tile_pool(name="vps", bufs=1, space="PSUM")
        lps = tc.alloc_tile_pool(name="lps", bufs=2, space="PSUM")
        dps = tc.alloc_tile_pool(name="dps", bufs=1, space="PSUM")
        apool = tc.alloc_tile_pool(name="aps", bufs=2, space="PSUM")
        fps = tc.alloc_tile_pool(name="fps", bufs=1, space="PSUM")
        yps = tc.alloc_tile_pool(name="yps", bufs=1, space="PSUM")

        xw_tiles = {}

        def chunk_rows(ci):
            t0 = GR0 + 3 * ci
            return t0, min(t0 + 3, GR1)

        def do_products(ci):
            t0, t1 = chunk_rows(ci)
            S0 = t0 * PW + 1
            SN = (t1 - t0) * PW - 2
            vbase = (t0 - 1) * PW
            vlen = (t1 - t0 + 2) * PW
            # vball regions: A [0:5PW): lower v, upper v<<1 (one col);
            #                B [5PW:10PW): lower v, upper v<<PW (one row).
            RB = 5 * PW
            vb = vpool.tile([128, 2 * RB], F32, tag="vb")
            for j in range(2):
                vo = j * 390
                vn = min(390, vlen - vo)
                if vn <= 0:
                    continue
                psv = vps.tile([64, 390], F32, tag="v")
                nc.tensor.matmul(psv[:, :vn], wr_at(wr, "wv", 0, 64),
                                 x2[0:64, vbase + vo:vbase + vo + vn],
                                 start=True, stop=True)
                nc.scalar.activation(vb[0:64, vo:vo + vn], psv[:, :vn],
                                     AF.Identity, bias=bvv)
                if vo == 0:
                    nc.scalar.activation(vb[64:128, 0:vn - 1], psv[:, 1:vn],
                                         AF.Identity, bias=bvv)
                else:
                    nc.scalar.activation(vb[64:128, vo - 1:vo - 1 + vn],
                                         psv[:, :vn], AF.Identity, bias=bvv)
            # region B from region A
            nc.scalar.copy(vb[0:64, RB:RB + vlen], vb[0:64, 0:vlen])
            nc.scalar.copy(vb[64:128, RB:RB + vlen - PW], vb[0:64, PW:vlen])
            edge = ci in (0, n_chunks - 2, n_chunks - 1)
            for p in range(9):
                psl = lps.tile([72, CONVN], F32, tag="l")
                nc.tensor.matmul(psl[:, :SN],
                                 wr[0:64, wcols["wa"] + p * 72:
                                    wcols["wa"] + p * 72 + 72],
                                 x2[0:64, S0:S0 + SN], start=True, stop=True)
                ep = epool.tile([72, CONVN], F32R, tag="e")
                nc.scalar.activation(ep[:, :SN], psl[:, :SN], AF.Exp,
                                     bias=bba(p))
                psd = dps.tile([72, CONVN], F32, tag="d")
                nc.tensor.matmul(psd[:, :SN], wr_at(wr, "den", 0, 72, 72),
                                 ep[:, :SN], start=True, stop=True)
                lse = lpool.tile([72, CONVN], F32, tag="lse")
                nc.scalar.activation(lse[:, :SN], psd[:, :SN], AF.Ln)
                sdf = spool2.tile([72, CONVN], F32, tag="sd")
                nc.vector.tensor_tensor(sdf[:, :SN], psl[:, :SN],
                                        lse[:, :SN], OP.subtract)
                sm = spool2.tile([72, CONVN], F32R, tag="sm")
                nc.scalar.activation(sm[:, :SN], sdf[:, :SN], AF.Exp)

                tM, tS = [], None
                for j in range(4):
                    psp = apool.tile([128, CONVN], F32, tag="a")
                    nc.tensor.matmul(psp[:, :SN],
                                     wr[0:72, wcols["selp"] + j * 128:
                                        wcols["selp"] + j * 128 + 128],
                                     sm[:, :SN], start=True, stop=True)
                    tm = tpool.tile([128, CONVN], BF16, tag=f"tM{j}")
                    if j < 3:
                        vlo = S0 - vbase + (j - 1) * PW - 1
                    else:
                        vlo = RB + S0 - vbase - PW + 1
                    nc.vector.tensor_tensor(tm[:, :SN], psp[:, :SN],
                                            vb[0:128, vlo:vlo + SN], OP.mult)
                    tM.append(tm)
                pss = apool.tile([128, CONVN], F32, tag="a")
                nc.tensor.matmul(pss[0:64, :SN], wr_at(wr, "sels8", 0, 72),
                                 sm[:, :SN], start=True, stop=True)
                tS = tpool.tile([64, CONVN], BF16, tag="tS")
                vso = S0 - vbase + PW + 1
                nc.vector.tensor_tensor(tS[:, :SN], pss[0:64, :SN],
                                        vb[0:64, vso:vso + SN], OP.mult)
                psf = fps.tile([64, CONVN], F32, tag="f")
                for j in range(4):
                    nc.tensor.matmul(psf[:, :SN], fold128[:, :],
                                     tM[j][:, :SN], start=(j == 0), stop=False)
                nc.tensor.matmul(psf[:, :SN], wb[0:64, 64:128],
                                 tS[:, :SN], start=False, stop=True)
                xw = xwpool.tile([64, 3 * PW], BF16, tag=f"xw{p}")
                nc.scalar.copy(xw[:, 1:1 + SN], psf[:, :SN])
                if edge:
                    x3 = xw[:].rearrange("c (r w) -> c r w", r=3, w=PW)
                    m = mskf[0:64, t0:t0 + 3].rearrange("c (r u) -> c r u", u=1)
                    a2, m2 = broadcast_tensor_aps(x3, m)
                    nc.vector.tensor_tensor(a2, a2, m2, OP.mult)
                xwf = xw[:].bitcast(mybir.dt.uint16)
                nc.gpsimd.memset(xwf[:, 0:1], 0.0)
                nc.gpsimd.memset(xwf[:, 3 * PW - 1:3 * PW], 0.0)
                if t1 - t0 < 3:
                    nc.gpsimd.memset(xwf[:, (t1 - t0) * PW:3 * PW], 0.0)
                xw_tiles[(p, ci)] = (xw, t0)

        def do_proj(ci):
            t0, t1 = chunk_rows(ci)
            zr0, zr1 = max(t0, ZR0), min(t1, ZR1)
            if zr0 >= zr1:
                return
            zS0 = zr0 * PW + 1
            zSN = (zr1 - zr0) * PW - 2
            mms = []
            for p, (di, dj) in enumerate(TAPS):
                off = di * PW + dj
                a, b = zS0 - off, zS0 - off + zSN
                for cj in (ci - 1, ci, ci + 1):
                    if (p, cj) not in xw_tiles:
                        continue
                    xw, xt0 = xw_tiles[(p, cj)]
                    lo, hi = xt0 * PW, xt0 * PW + 3 * PW
                    pa, pb = max(a, lo), min(b, hi)
                    if pa >= pb:
                        continue
                    mms.append((pa - a, pb - a, xw, pa - lo, pb - lo))
            psy = yps.tile([64, CONVN], F32, tag="y")
            for i, (oa, ob, xw, xa, xb) in enumerate(mms):
                nc.tensor.matmul(psy[:, oa:ob], wb[0:64, 0:64],
                                 xw[0:64, xa:xb],
                                 start=(i == 0), stop=(i == len(mms) - 1))
            nc.scalar.activation(z[0:64, zS0:zS0 + zSN], psy[:, :zSN],
                                 AF.Identity, bias=bpp)
            if zr0 < 7 or zr1 > 71:
                # mask whole rows [zr0, zr1); skip partial first/last cols
                # (those lie in the padded col region, zeroed below)
                mask_rows(z, zr0, zr1)

        for ci in range(n_chunks):
            do_products(ci)
            if ci >= 1:
                do_proj(ci - 1)
        do_proj(n_chunks - 1)
        z3 = z[0:64].bitcast(F32).rearrange("c (r w) -> c r w", r=TROWS, w=PW)
        nc.gpsimd.memset(z3[:, ZR0:ZR1, 0:1], 0.0)
        nc.gpsimd.memset(z3[:, ZR0:ZR1, PW - 1:PW], 0.0)

        for pool in (yps, fps, apool, dps, lps, vps, xwpool, tpool, spool2,
                     lpool, epool, vpool):
            pool.release()

        # ---- convs 3,4 and output ----
        y3 = spool.tile([64, NPX], F32R, tag="stage")
        with tc.tile_pool(name="cps2", bufs=4, space="PSUM") as cps2:
            conv_stage(cps2, z, y3, 2, 6, 72)
            y4 = spool.tile([64, NPX], F32, tag="stage")
            conv_stage(cps2, y3, y4, 3, 7, 71)
        ysrc = y4[0:64].rearrange("c (r w) -> c r w", r=TROWS, w=PW)
        nc.sync.dma_start(y[:].rearrange("c (r w) -> c r w", r=64, w=W),
                          ysrc[:, 7:71, 1:1 + W])
        spool.release()
        cpool.release()
    return nc


_CACHE = {}


def kernel(**inputs):
    x = np.asarray(inputs["x"], np.float32)
    conv_w = np.asarray(inputs["conv_w"], np.float32)
    args = (conv_w, np.asarray(inputs["bn_g"], np.float32),
            np.asarray(inputs["bn_b"], np.float32),
            np.asarray(inputs["bn_m"], np.float32),
            np.asarray(inputs["bn_v"], np.float32),
            np.asarray(inputs["w_v"], np.float32),
            np.asarray(inputs["b_v"], np.float32),
            np.asarray(inputs["w_attn"], np.float32),
            np.asarray(inputs["b_attn"], np.float32),
            np.asarray(inputs["w_proj"], np.float32),
            np.asarray(inputs["b_proj"], np.float32))
    wts, wcols, fold = _prep_weights(*args)

    # per-core input shards + row-validity masks
    shards, masks = [], []
    for core in range(N_CORES):
        bb, half = divmod(core, 2)
        s = half * 64
        rowbase = s - 7
        sh = np.zeros((64, TROWS, PW), np.float32)
        r0, r1 = max(0, rowbase + 1), min(H, rowbase + 77)
        sh[:, r0 - rowbase:r1 - rowbase, 1:1 + W] = x[bb, :, r0:r1, :]
        shards.append(sh.reshape(64, NPX))
        mk = np.zeros((72, TROWS), np.float32)
        rr = np.arange(TROWS) + rowbase
        mk[:, (rr >= 0) & (rr < H)] = 1.0
        masks.append(mk)

    key = "k2"
    if key not in _CACHE:
        _CACHE[key] = _build(wcols, wts.shape[1])
    nc = _CACHE[key]
    in_maps = [{"xs": shards[i], "wts": wts, "wts2": fold, "msk": masks[i]}
               for i in range(N_CORES)]
    tkw = {}
    if os.environ.get("KERNEL_TRACE"):
        tkw = dict(trace=True, tmpdir=os.environ.get("KERNEL_TRACE_DIR"))
    res = run_bass_kernel_spmd(nc, in_maps, core_ids=list(range(N_CORES)),
                               **tkw)
    global LAST_RESULT
    LAST_RESULT = res
    out = np.zeros((B, C, H, W), np.float32)
    for core in range(N_CORES):
        bb, half = divmod(core, 2)
        s = half * 64
        out[bb, :, s:s + 64, :] = res.results[core]["y"].reshape(C, 64, W)
    return out
